# revision 19
# baseline (speedup 1.0000x reference)
"""Expert-parallel MoE FFN kernel for Trainium2 (8 NeuronCores).

Problem: inputs [4, 8192, 1024], per-expert FFN with E=8 experts:
  x -> x @ w1[e].T + b1[e] -> gelu -> @ w2[e].T + b2[e]
Sharding: expert-parallel, one expert per core (DeepSpeed expert-parallel
layout). No collectives needed: core e gets inputs[:, e*C:(e+1)*C, :] and
expert e's weights, produces that slice of the output.

Per-core compute: [4096,1024] @ [1024,4096] -> gelu -> @ [4096,1024]
(68.7 GFLOP). Matmul operands are fp16 (PSUM accumulation stays fp32):
full PE rate with the LDWEIGHTS hidden behind the moving stream, unlike
fp32 (4 cycles/row) or float32r (whose mandatory self-loading weight
fetch serializes ~107ns per matmul). Measured ~910us/core = 97% of the
4096-matmul N=512 floor; scale-relative max error ~4.4e-4.

Device layout (all transposes + tiling done host-side, free):
  phase 1: hT[f, t] = gelu(w1T[d, f].T @ xT[d, t] + b1[f])   (K=d on partitions)
  phase 2: yT[d, t] = w2T[f, d].T @ hT[f, t] + b2[d]          (K=f on partitions)
Host untransposes yT -> y. DRAM tensors are pre-packed so every SBUF
tile fills with a single contiguous dma_start (the Sync queue saturates
near ~2000 descriptor issues otherwise), and DMA issue is spread across
the Sync/Activation/GpSimd queues.
"""

import time

import numpy as np

import concourse.bacc as bacc
import concourse.mybir as mybir
import concourse.tile as tile
from concourse.bass_utils import run_bass_kernel_spmd
from concourse.mybir import ActivationFunctionType as AFT

E = 8          # experts == cores
D = 1024       # d_model
F = 4096       # d_ff
B, C = 4, 1024
T = B * C      # tokens per expert (4096)
TT = 1024      # token tile
NTT = T // TT  # 4
FBW = 1024     # f-block width
NFB = F // FBW # 4
KD = D // 128  # 8 k-chunks over d
KF = FBW // 128  # 8 f-chunks per f-block
ND = D // 128  # 8 d-chunks
f32 = mybir.dt.float32
f32r = mybir.dt.float32r
f16 = mybir.dt.float16

_COMPILED = None  # (nc, input_names)


def _build():
    nc = bacc.Bacc("TRN2", target_bir_lowering=False, debug=False)

    xt_d = nc.dram_tensor("xt", [NTT, KD, 128, TT], f16, kind="ExternalInput")
    w1_d = nc.dram_tensor("w1", [F // 128, 128, KD * 128], f16, kind="ExternalInput")
    w2_d = nc.dram_tensor("w2", [NFB, ND, 128, KF * 128], f16, kind="ExternalInput")
    b1_d = nc.dram_tensor("b1", [128, F // 128], f32, kind="ExternalInput")
    b2_d = nc.dram_tensor("b2", [128, ND], f32, kind="ExternalInput")
    yt_d = nc.dram_tensor("yt", [D, T], f32, kind="ExternalOutput")

    xt = xt_d.ap()
    w1 = w1_d.ap()
    w2 = w2_d.ap()
    yt = yt_d.ap()

    with tile.TileContext(nc) as tc:
        with (
            tc.tile_pool(name="xp", bufs=3) as xp,
            tc.tile_pool(name="w1p", bufs=8) as w1p,
            tc.tile_pool(name="w2p", bufs=8) as w2p,
            tc.tile_pool(name="hp", bufs=2) as hp,
            tc.tile_pool(name="yp", bufs=2) as yp,
            tc.tile_pool(name="bp", bufs=1) as bp,
            tc.tile_pool(name="hpp", bufs=2, space="PSUM") as hpp,
            tc.tile_pool(name="ypp", bufs=4, space="PSUM") as ypp,
        ):
            b1_sb = bp.tile([128, F // 128], f32, tag="b1")
            nc.scalar.dma_start(b1_sb[:], b1_d.ap()[:])
            b2_sb = bp.tile([128, ND], f32, tag="b2")
            nc.scalar.dma_start(b2_sb[:], b2_d.ap()[:])

            for tt in range(NTT):
                xks = []
                for k in range(KD):
                    xk = xp.tile([128, TT], f16, tag=f"xk{k}", name=f"xk_{k}")
                    nc.gpsimd.dma_start(xk[:], xt[tt, k])
                    xks.append(xk)
                yacc = yp.tile([128, ND * TT], f32, tag="yacc")

                for fb in range(NFB):
                    htile = hp.tile([128, KF * TT], f16, tag="h")
                    # ---- phase 1: hT[fb] = gelu(w1T.T @ xT + b1) ----
                    for fc in range(KF):
                        g = fb * KF + fc
                        w1t = w1p.tile([128, KD * 128], f16, tag="w1")
                        nc.sync.dma_start(w1t[:], w1[g])
                        ph = hpp.tile([128, TT], f32, tag="hps")
                        for k in range(KD):
                            for th in range(TT // 512):
                                nc.tensor.matmul(
                                    ph[:, th * 512:(th + 1) * 512],
                                    w1t[:, k * 128:(k + 1) * 128],
                                    xks[k][:, th * 512:(th + 1) * 512],
                                    start=(k == 0),
                                    stop=(k == KD - 1),
                                )
                        nc.scalar.activation(
                            htile[:, fc * TT:(fc + 1) * TT], ph[:],
                            AFT.Gelu, bias=b1_sb[:, g:g + 1],
                        )

                    # ---- phase 2: yT += w2T.T @ hT[fb] (+ b2 on first block) ----
                    for dcg in range(ND // 2):
                        w2ts = []
                        for j in range(2):
                            dc = dcg * 2 + j
                            w2t = w2p.tile([128, KF * 128], f16, tag="w2")
                            nc.sync.dma_start(w2t[:], w2[fb, dc])
                            w2ts.append(w2t)
                        pys = [
                            ypp.tile([128, 512], f32, tag="yps", name=f"yps_{i}")
                            for i in range(2 * (TT // 512))
                        ]
                        for fc in range(KF):
                            for j in range(2):
                                for th in range(TT // 512):
                                    nc.tensor.matmul(
                                        pys[j * (TT // 512) + th][:],
                                        w2ts[j][:, fc * 128:(fc + 1) * 128],
                                        htile[:, fc * TT + th * 512:fc * TT + (th + 1) * 512],
                                        start=(fc == 0),
                                        stop=(fc == KF - 1),
                                    )  # j-major keeps w2 stationary across th

                        for j in range(2):
                            dc = dcg * 2 + j
                            for th in range(TT // 512):
                                dst = yacc[:, dc * TT + th * 512:dc * TT + (th + 1) * 512]
                                py = pys[j * (TT // 512) + th][:]
                                if fb == 0:
                                    nc.scalar.activation(
                                        dst, py, AFT.Identity, bias=b2_sb[:, dc:dc + 1]
                                    )
                                else:
                                    nc.vector.tensor_add(dst, dst, py)
                            if fb == NFB - 1:
                                # final value for this dc: overlap the store
                                # with the remaining dcg compute; alternate
                                # queues so the last two stores overlap
                                eng = nc.scalar if dc % 2 == 0 else nc.sync
                                eng.dma_start(
                                    yt[dc * 128:(dc + 1) * 128, tt * TT:(tt + 1) * TT],
                                    yacc[:, dc * TT:(dc + 1) * TT],
                                )

    nc.compile()
    return nc


def _get_compiled():
    global _COMPILED
    if _COMPILED is None:
        _COMPILED = _build()
    return _COMPILED


def _pack_core(x_e, w1_e, b1_e, w2_e, b2_e):
    """Host-side repack of one expert's tensors into the kernel's tiled layouts."""
    xT = x_e.reshape(T, D).T                      # [D, T]
    xt = np.ascontiguousarray(
        xT.reshape(KD, 128, NTT, TT).transpose(2, 0, 1, 3)
    ).astype(np.float16)                          # [NTT, KD, 128, TT]
    w1T = w1_e.T                                  # [D, F]
    w1t = np.ascontiguousarray(
        w1T.reshape(KD, 128, F // 128, 128).transpose(2, 1, 0, 3).reshape(F // 128, 128, KD * 128)
    ).astype(np.float16)                          # [F//128, 128, KD*128]
    w2T = w2_e.T                                  # [F, D]
    w2t = np.ascontiguousarray(
        w2T.reshape(NFB, KF, 128, ND, 128).transpose(0, 3, 2, 1, 4).reshape(NFB, ND, 128, KF * 128)
    ).astype(np.float16)                          # [NFB, ND, 128, KF*128]
    b1t = np.ascontiguousarray(b1_e.reshape(F // 128, 128).T)  # [128, F//128]
    b2t = np.ascontiguousarray(b2_e.reshape(ND, 128).T)        # [128, ND]
    return {"xt": xt, "w1": w1t, "w2": w2t, "b1": b1t, "b2": b2t}


def kernel(inputs, w1, b1, w2, b2):
    inputs = np.asarray(inputs, dtype=np.float32)
    w1 = np.asarray(w1, dtype=np.float32)
    b1 = np.asarray(b1, dtype=np.float32)
    w2 = np.asarray(w2, dtype=np.float32)
    b2 = np.asarray(b2, dtype=np.float32)

    nc = _get_compiled()

    in_maps = []
    for e in range(E):
        x_e = inputs[:, e * C:(e + 1) * C, :]     # [B, C, D]
        in_maps.append(_pack_core(x_e, w1[e], b1[e], w2[e], b2[e]))

    # The axon-tunneled devices occasionally come up wedged
    # (NRT_EXEC_UNIT_UNRECOVERABLE on the first execute); a retry after a
    # short pause reliably recovers.
    last_err = None
    for attempt in range(3):
        try:
            res = run_bass_kernel_spmd(nc, in_maps, core_ids=list(range(E)))
            out = np.empty((B, E * C, D), dtype=np.float32)
            for e in range(E):
                yT = np.asarray(res.results[e]["yt"])  # [D, T]
                out[:, e * C:(e + 1) * C, :] = yT.T.reshape(B, C, D)
            return out
        except Exception as err:  # noqa: BLE001 - device flake, retry
            last_err = err
            time.sleep(10 * (attempt + 1))
    raise last_err
